# revision 23
# baseline (speedup 1.0000x reference)
"""Trainium2 Bass kernel for the thin-plate-spline RBF layer.

reference:  out[b,n,d] = sum_m phi(|x_bn - c_bm|) * w[b,m,d],
            phi(r) = r^2 * log(r + 1e-6)

Device algorithm (per core, N sharded 8 ways):
  dist2[m,n] = sum_k a_k[m] * b_k[n]   -- rank-15 bf16 split-precision
      expansion of |x-c|^2 (coordinates centered, split into bf16 hi/lo;
      bf16 products are exact under fp32 PSUM accumulation).
      The four batches are packed into the 128x128 PE array as four
      32-row strips (tile_position row tiling) and run concurrently.
  L[m,n] = ln(dist2 + 5e-5)            (ScalarE)
  The elementwise dist2*L multiply is eliminated algebraically:
    out[b,n,d] = sum_k b_k[n] * S[(k,b,d), n],
    S = sum_m (0.5 * a_k[m] * w[m,d]) * L[m,n]   (TensorE fp32r,
        batch-stacked block-diagonal weights, 60 columns).
  Two consecutive n-tiles are computed concurrently via column tiling:
  chain A in array columns 0-59 / PSUM rows 0-59, chain B in columns
  64-123 / rows 64-123.  Same pairing for the 0/1 k-reduction matrix R
  (row-packed at partition bases 0 and 64).
"""
import sys

sys.path.insert(0, "/opt/trn_rl_repo")

import numpy as np
import ml_dtypes

BF16 = np.dtype(ml_dtypes.bfloat16)

B, M, N, NCORES = 4, 256, 32768, 8
NS = N // NCORES          # 4096 dense points per core
NT = 512                  # n-tile (one PSUM bank of fp32)
NTILES = NS // NT         # 8
NPAIRS = NTILES // 2      # 4 column-paired tile groups
HALVES = M // 128         # 2
NBLK = B * HALVES         # 8 contraction blocks of 128
KD = 15                   # dist2 split-precision rank
J = 5 * B * 3             # 60 stacked S columns, j = k*12 + b*3 + d
JP = 64                   # padded to one column-tile quadrant
DELTA = 5e-5

_compiled = None


def _build_nc():
    import concourse.bacc as bacc
    import concourse.mybir as mybir
    from concourse.tile import TileContext

    f32 = mybir.dt.float32
    f32r = mybir.dt.float32r
    bf = mybir.dt.bfloat16
    f16 = mybir.dt.float16
    nc = bacc.Bacc("TRN2")

    daug_d = nc.dram_tensor("daug", [128, NS], bf, kind="ExternalInput")
    bcs_d = nc.dram_tensor("bcs", [J, NS], f32, kind="ExternalInput")
    cpa_d = nc.dram_tensor("cpa", [128, HALVES * 128], bf, kind="ExternalInput")
    wps_d = nc.dram_tensor("wps", [NBLK, 128, JP], f32r, kind="ExternalInput")
    rmat_d = nc.dram_tensor("rmat", [128, 64], f32r, kind="ExternalInput")
    out_d = nc.dram_tensor("outb", [12, NS], f32, kind="ExternalOutput")

    with TileContext(nc) as tc:
        with (
            tc.tile_pool(name="singles", bufs=1) as singles,
            tc.tile_pool(name="lpool", bufs=34) as lpool,
            tc.tile_pool(name="zpool", bufs=3) as zpool,
            tc.tile_pool(name="d2pool", bufs=2, space="PSUM") as d2pool,
            tc.tile_pool(name="spool", bufs=2, space="PSUM") as spool,
            tc.tile_pool(name="opool", bufs=2, space="PSUM") as opool,
        ):
            delta_t = singles.tile([128, 1], f32)
            nc.vector.memset(delta_t, DELTA)

            # inputs spread across DGE paths; most-urgent first
            scratch = singles.tile([128, NT], bf)
            nc.gpsimd.memset(scratch[:], 0.0)
            cpa_t = singles.tile([128, HALVES * 128], bf)
            nc.gpsimd.dma_start(out=cpa_t[:], in_=cpa_d[:])
            daug_t = singles.tile([128, NS], bf)
            nc.sync.dma_start(out=daug_t[:, : NS // 2], in_=daug_d[:, : NS // 2])
            nc.sync.dma_start(out=daug_t[:, NS // 2 :], in_=daug_d[:, NS // 2 :])
            wps_t = []
            for l in range(NBLK):
                t = singles.tile([128, JP], f32r, tag=f"wps{l}")
                nc.gpsimd.dma_start(out=t[:], in_=wps_d[l])
                wps_t.append(t)
            rmat_t = singles.tile([128, 64], f32r)
            nc.gpsimd.dma_start(out=rmat_t[:], in_=rmat_d[:])
            bcs_t = singles.tile([J, NS], f32)
            nc.gpsimd.dma_start(out=bcs_t[:, : NS // 2], in_=bcs_d[:, : NS // 2])
            nc.gpsimd.dma_start(out=bcs_t[:, NS // 2 :], in_=bcs_d[:, NS // 2 :])
            # (cpa + first daug half land first: they gate the first matmul)
            out_sb = singles.tile([12, NS], f32)

            # HAM warmup on junk data while input DMAs land
            wtile = d2pool.tile([128, 2 * NT], f32, tag="d2")
            for _ in range(12):
                nc.tensor.matmul(
                    wtile[:, : NT // 2], scratch[:, :128], scratch[:, : NT // 2],
                    start=True, stop=True,
                )

            # ---- phase 1 emission: all dist2 matmuls + ln ----
            # d2/L tiles hold [b_even | b_odd] for one (h, n-tile); the two
            # dist2 matmuls go to different PE row strips (tile_position)
            # and different PSUM banks, so they run concurrently.
            ltiles = {}
            for nt_ in range(NTILES):
                nsl = slice(nt_ * NT, (nt_ + 1) * NT)
                for h in range(HALVES):
                    for i in range(2):          # batch pair {2i, 2i+1}
                        d2 = d2pool.tile([128, 2 * NT], f32, tag="d2")
                        for bi in range(2):
                            b = 2 * i + bi
                            nc.tensor.matmul(
                                d2[:, bi * NT : (bi + 1) * NT],
                                cpa_t[32 * b : 32 * b + KD,
                                      h * 128 : (h + 1) * 128],
                                daug_t[32 * b : 32 * b + KD, nsl],
                                start=True,
                                stop=True,
                                tile_position=(32 * b, 0),
                            )
                        lt = lpool.tile([128, 2 * NT], f32r, tag="L")
                        nc.scalar.activation(
                            out=lt[:],
                            in_=d2[:],
                            func=mybir.ActivationFunctionType.Ln,
                            bias=delta_t[:],
                            scale=1.0,
                        )
                        ltiles[(nt_, h, i)] = lt

            # ---- phase 2 emission: S chains, combine, reduce, store ----
            for P in range(NPAIRS):
                nsl_a = slice((2 * P) * NT, (2 * P + 1) * NT)
                nsl_b = slice((2 * P + 1) * NT, (2 * P + 2) * NT)
                svals = []
                for ab in range(2):
                    nt_ = 2 * P + ab
                    s_c = spool.tile([64, NT], f32, tag="S")
                    for l in range(NBLK):
                        b, h = l // 2, l % 2
                        lt = ltiles[(nt_, h, b // 2)]
                        col = (b % 2) * NT
                        nc.tensor.matmul(
                            s_c[:],
                            wps_t[l][:],
                            lt[:, col : col + NT],
                            start=(l == 0),
                            stop=(l == NBLK - 1),
                        )
                    svals.append(s_c)

                z_a = zpool.tile([J, NT], f32r, tag="z")
                z_b = zpool.tile([J, NT], f32r, tag="z")
                nc.vector.tensor_mul(z_a[:], svals[0][0:J, :], bcs_t[:, nsl_a])
                nc.vector.tensor_mul(z_b[:], svals[1][0:J, :], bcs_t[:, nsl_b])

                o2a = opool.tile([12, NT], f32, tag="o2")
                o2b = opool.tile([12, NT], f32, tag="o2")
                nc.tensor.matmul(o2a[:], rmat_t[0:J, 0:12], z_a[:],
                                 start=True, stop=True)
                nc.tensor.matmul(o2b[:], rmat_t[0:J, 0:12], z_b[:],
                                 start=True, stop=True)
                nc.vector.tensor_copy(out_sb[:, nsl_a], o2a[:])
                nc.vector.tensor_copy(out_sb[:, nsl_b], o2b[:])
                nc.sync.dma_start(
                    out=out_d[:, (2 * P) * NT : (2 * P + 2) * NT],
                    in_=out_sb[:, (2 * P) * NT : (2 * P + 2) * NT],
                )

    nc.compile()
    return nc


def _split3(v):
    """3-way bf16 split of float64 array."""
    hi = v.astype(BF16)
    r1 = v - hi.astype(np.float64)
    mid = r1.astype(BF16)
    r2 = r1 - mid.astype(np.float64)
    lo = r2.astype(BF16)
    return hi, mid, lo


def _host_prep(sparse_disp, original_cp, original_dense):
    """Build per-core input maps for the device kernel."""
    x = original_dense.astype(np.float64) - 0.5   # (B, N, 3) centered
    c = original_cp.astype(np.float64) - 0.5      # (B, M, 3)
    w = sparse_disp.astype(np.float32)            # (B, M, 3)

    # ---- control-point side (shared by all cores) ----
    p = c.astype(BF16)
    q = (c - p.astype(np.float64)).astype(BF16)
    t_hi, t_mid, t_lo = _split3((c * c).sum(-1))
    ones_m = np.ones((B, M), BF16)

    # per-batch KD rows: [p x3, p x3, q x3, t_hi, t_mid, t_lo, 1, 1, 1]
    cpa_full = np.empty((B, KD, M), BF16)
    for d in range(3):
        cpa_full[:, d, :] = p[:, :, d]
        cpa_full[:, 3 + d, :] = p[:, :, d]
        cpa_full[:, 6 + d, :] = q[:, :, d]
    cpa_full[:, 9, :] = t_hi
    cpa_full[:, 10, :] = t_mid
    cpa_full[:, 11, :] = t_lo
    cpa_full[:, 12, :] = ones_m
    cpa_full[:, 13, :] = ones_m
    cpa_full[:, 14, :] = ones_m

    # stacked stationary: rows 32b..32b+KD, cols h*128..
    cpa = np.zeros((128, HALVES * 128), BF16)
    for b in range(B):
        for h in range(HALVES):
            cpa[32 * b : 32 * b + KD, h * 128 : (h + 1) * 128] = \
                cpa_full[b, :, h * 128 : (h + 1) * 128]

    wps = np.zeros((NBLK, 128, JP), np.float32)  # cast to bf16 at the end
    c32 = c.astype(np.float32)
    a5 = np.stack(
        [c32[:, :, 0], c32[:, :, 1], c32[:, :, 2],
         (c32 * c32).sum(-1), np.ones((B, M), np.float32)],
        axis=1,
    )  # (B, 5, M)
    for b in range(B):
        for h in range(HALVES):
            l = 2 * b + h
            msl = slice(h * 128, (h + 1) * 128)
            for k in range(5):
                for d in range(3):
                    j = k * 12 + b * 3 + d
                    wps[l, :, j] = 0.5 * a5[b, k, msl] * w[b, msl, d]

    rmat = np.zeros((128, 64), np.float32)
    for j in range(J):
        rmat[j, j % 12] = 1.0
        rmat[64 + j, j % 12] = 1.0

    # ---- dense-point side (per core) ----
    u_all = x.astype(BF16)
    v_all = (x - u_all.astype(np.float64)).astype(BF16)
    s_all = (x * x).sum(-1)

    in_maps = []
    for core in range(NCORES):
        csl = slice(core * NS, (core + 1) * NS)
        u = u_all[:, csl, :].astype(np.float32)
        v = v_all[:, csl, :].astype(np.float32)
        s_hi, s_mid, s_lo = _split3(s_all[:, csl])
        ones_n = np.ones((B, NS), BF16)

        daug_b = np.empty((B, KD, NS), BF16)
        for d in range(3):
            daug_b[:, d, :] = (-2.0 * u[:, :, d]).astype(BF16)
            daug_b[:, 3 + d, :] = (-2.0 * v[:, :, d]).astype(BF16)
            daug_b[:, 6 + d, :] = (-2.0 * u[:, :, d]).astype(BF16)
        daug_b[:, 9, :] = ones_n
        daug_b[:, 10, :] = ones_n
        daug_b[:, 11, :] = ones_n
        daug_b[:, 12, :] = s_hi
        daug_b[:, 13, :] = s_mid
        daug_b[:, 14, :] = s_lo

        daug = np.zeros((128, NS), BF16)
        for b in range(B):
            daug[32 * b : 32 * b + KD] = daug_b[b]

        xs = x[:, csl, :].astype(np.float32)
        baug5 = np.stack(
            [-2.0 * xs[:, :, 0], -2.0 * xs[:, :, 1], -2.0 * xs[:, :, 2],
             np.ones((B, NS), np.float32), (xs * xs).sum(-1)],
            axis=1,
        )  # (B, 5, NS)
        bc = np.empty((J, NS), np.float32)
        for k in range(5):
            for b in range(B):
                for d in range(3):
                    bc[k * 12 + b * 3 + d] = baug5[b, k]

        in_maps.append(
            {
                "daug": daug,
                "bcs": bc,
                "cpa": cpa,
                "wps": wps,
                "rmat": rmat,
            }
        )
    return in_maps


def _assemble(results):
    out = np.empty((B, N, 3), np.float32)
    for core, r in enumerate(results):
        o = r["outb"]  # (12, NS) rows b*3+d
        out[:, core * NS : (core + 1) * NS, :] = (
            o.reshape(B, 3, NS).transpose(0, 2, 1)
        )
    return out


def kernel(sparse_disp, original_cp, original_dense):
    global _compiled
    from concourse.bass_utils import run_bass_kernel_spmd

    if _compiled is None:
        _compiled = _build_nc()
    in_maps = _host_prep(sparse_disp, original_cp, original_dense)
    res = run_bass_kernel_spmd(_compiled, in_maps, core_ids=list(range(NCORES)))
    return _assemble(res.results)


# revision 24
# speedup vs baseline: 1.0108x; 1.0108x over previous
"""Trainium2 Bass kernel for the thin-plate-spline RBF layer.

reference:  out[b,n,d] = sum_m phi(|x_bn - c_bm|) * w[b,m,d],
            phi(r) = r^2 * log(r + 1e-6)

Device algorithm (per core, N sharded 8 ways):
  dist2[m,n] = sum_k a_k[m] * b_k[n]   -- rank-15 bf16 split-precision
      expansion of |x-c|^2 (coordinates centered, split into bf16 hi/lo;
      bf16 products are exact under fp32 PSUM accumulation).
      The four batches are packed into the 128x128 PE array as four
      32-row strips (tile_position row tiling) and run concurrently.
  L[m,n] = ln(dist2 + 5e-5)            (ScalarE)
  The elementwise dist2*L multiply is eliminated algebraically:
    out[b,n,d] = sum_k b_k[n] * S[(k,b,d), n],
    S = sum_m (0.5 * a_k[m] * w[m,d]) * L[m,n]   (TensorE fp32r,
        batch-stacked block-diagonal weights, 60 columns).
  Two consecutive n-tiles are computed concurrently via column tiling:
  chain A in array columns 0-59 / PSUM rows 0-59, chain B in columns
  64-123 / rows 64-123.  Same pairing for the 0/1 k-reduction matrix R
  (row-packed at partition bases 0 and 64).
"""
import sys

sys.path.insert(0, "/opt/trn_rl_repo")

import numpy as np
import ml_dtypes

BF16 = np.dtype(ml_dtypes.bfloat16)

B, M, N, NCORES = 4, 256, 32768, 8
NS = N // NCORES          # 4096 dense points per core
NT = 512                  # n-tile (one PSUM bank of fp32)
NTILES = NS // NT         # 8
NPAIRS = NTILES // 2      # 4 column-paired tile groups
HALVES = M // 128         # 2
NBLK = B * HALVES         # 8 contraction blocks of 128
KD = 15                   # dist2 split-precision rank
J = 5 * B * 3             # 60 stacked S columns, j = k*12 + b*3 + d
JP = 64                   # padded to one column-tile quadrant
DELTA = 5e-5

_compiled = None


def _build_nc():
    import concourse.bacc as bacc
    import concourse.mybir as mybir
    from concourse.tile import TileContext

    f32 = mybir.dt.float32
    f32r = mybir.dt.float32r
    bf = mybir.dt.bfloat16
    f16 = mybir.dt.float16
    nc = bacc.Bacc("TRN2")

    daug_d = nc.dram_tensor("daug", [128, NS], bf, kind="ExternalInput")
    bcs_d = nc.dram_tensor("bcs", [J, NS], f32, kind="ExternalInput")
    cpa_d = nc.dram_tensor("cpa", [128, HALVES * 128], bf, kind="ExternalInput")
    wps_d = nc.dram_tensor("wps", [NBLK, 128, JP], f32r, kind="ExternalInput")
    rmat_d = nc.dram_tensor("rmat", [128, 64], f32r, kind="ExternalInput")
    out_d = nc.dram_tensor("outb", [12, NS], f32, kind="ExternalOutput")

    with TileContext(nc) as tc:
        with (
            tc.tile_pool(name="singles", bufs=1) as singles,
            tc.tile_pool(name="lpool", bufs=34) as lpool,
            tc.tile_pool(name="zpool", bufs=3) as zpool,
            tc.tile_pool(name="d2pool", bufs=2, space="PSUM") as d2pool,
            tc.tile_pool(name="spool", bufs=3, space="PSUM") as spool,
            tc.tile_pool(name="opool", bufs=1, space="PSUM") as opool,
        ):
            delta_t = singles.tile([128, 1], f32)
            nc.vector.memset(delta_t, DELTA)

            # inputs spread across DGE paths; most-urgent first
            scratch = singles.tile([128, NT], bf)
            nc.gpsimd.memset(scratch[:], 0.0)
            cpa_t = singles.tile([128, HALVES * 128], bf)
            nc.gpsimd.dma_start(out=cpa_t[:], in_=cpa_d[:])
            daug_t = singles.tile([128, NS], bf)
            nc.sync.dma_start(out=daug_t[:, : NS // 2], in_=daug_d[:, : NS // 2])
            nc.sync.dma_start(out=daug_t[:, NS // 2 :], in_=daug_d[:, NS // 2 :])
            wps_t = []
            for l in range(NBLK):
                t = singles.tile([128, JP], f32r, tag=f"wps{l}")
                nc.gpsimd.dma_start(out=t[:], in_=wps_d[l])
                wps_t.append(t)
            rmat_t = singles.tile([128, 64], f32r)
            nc.gpsimd.dma_start(out=rmat_t[:], in_=rmat_d[:])
            bcs_t = singles.tile([J, NS], f32)
            nc.gpsimd.dma_start(out=bcs_t[:, : NS // 2], in_=bcs_d[:, : NS // 2])
            nc.gpsimd.dma_start(out=bcs_t[:, NS // 2 :], in_=bcs_d[:, NS // 2 :])
            # (cpa + first daug half land first: they gate the first matmul)
            out_sb = singles.tile([12, NS], f32)

            # HAM warmup on junk data while input DMAs land
            wtile = d2pool.tile([128, 2 * NT], f32, tag="d2")
            for _ in range(10):
                nc.tensor.matmul(
                    wtile[:, : NT // 2], scratch[:, :128], scratch[:, : NT // 2],
                    start=True, stop=True,
                )

            # ---- phase 1 emission: all dist2 matmuls + ln ----
            # d2/L tiles hold [b_even | b_odd] for one (h, n-tile); the two
            # dist2 matmuls go to different PE row strips (tile_position)
            # and different PSUM banks, so they run concurrently.
            ltiles = {}
            for nt_ in range(NTILES):
                nsl = slice(nt_ * NT, (nt_ + 1) * NT)
                for h in range(HALVES):
                    for i in range(2):          # batch pair {2i, 2i+1}
                        d2 = d2pool.tile([128, 2 * NT], f32, tag="d2")
                        for bi in range(2):
                            b = 2 * i + bi
                            nc.tensor.matmul(
                                d2[:, bi * NT : (bi + 1) * NT],
                                cpa_t[32 * b : 32 * b + KD,
                                      h * 128 : (h + 1) * 128],
                                daug_t[32 * b : 32 * b + KD, nsl],
                                start=True,
                                stop=True,
                                tile_position=(32 * b, 0),
                            )
                        lt = lpool.tile([128, 2 * NT], f32r, tag="L")
                        nc.scalar.activation(
                            out=lt[:],
                            in_=d2[:],
                            func=mybir.ActivationFunctionType.Ln,
                            bias=delta_t[:],
                            scale=1.0,
                        )
                        ltiles[(nt_, h, i)] = lt

            # ---- phase 2 emission: S chains, combine, reduce, store ----
            for nt_ in range(NTILES):
                nsl = slice(nt_ * NT, (nt_ + 1) * NT)
                s_c = spool.tile([64, NT], f32, tag="S")
                for l in range(NBLK):
                    b, h = l // 2, l % 2
                    lt = ltiles[(nt_, h, b // 2)]
                    col = (b % 2) * NT
                    nc.tensor.matmul(
                        s_c[:],
                        wps_t[l][:],
                        lt[:, col : col + NT],
                        start=(l == 0),
                        stop=(l == NBLK - 1),
                    )
                z_t = zpool.tile([J, NT], f32r, tag="z")
                nc.vector.tensor_mul(z_t[:], s_c[0:J, :], bcs_t[:, nsl])
                o2 = opool.tile([12, NT], f32, tag="o2")
                nc.tensor.matmul(o2[:], rmat_t[0:J, 0:12], z_t[:],
                                 start=True, stop=True)
                nc.vector.tensor_copy(out_sb[:, nsl], o2[:])
                if nt_ % 2 == 1:
                    osl = slice((nt_ - 1) * NT, (nt_ + 1) * NT)
                    nc.sync.dma_start(out=out_d[:, osl], in_=out_sb[:, osl])

    nc.compile()
    return nc


def _split3(v):
    """3-way bf16 split of float64 array."""
    hi = v.astype(BF16)
    r1 = v - hi.astype(np.float64)
    mid = r1.astype(BF16)
    r2 = r1 - mid.astype(np.float64)
    lo = r2.astype(BF16)
    return hi, mid, lo


def _host_prep(sparse_disp, original_cp, original_dense):
    """Build per-core input maps for the device kernel."""
    x = original_dense.astype(np.float64) - 0.5   # (B, N, 3) centered
    c = original_cp.astype(np.float64) - 0.5      # (B, M, 3)
    w = sparse_disp.astype(np.float32)            # (B, M, 3)

    # ---- control-point side (shared by all cores) ----
    p = c.astype(BF16)
    q = (c - p.astype(np.float64)).astype(BF16)
    t_hi, t_mid, t_lo = _split3((c * c).sum(-1))
    ones_m = np.ones((B, M), BF16)

    # per-batch KD rows: [p x3, p x3, q x3, t_hi, t_mid, t_lo, 1, 1, 1]
    cpa_full = np.empty((B, KD, M), BF16)
    for d in range(3):
        cpa_full[:, d, :] = p[:, :, d]
        cpa_full[:, 3 + d, :] = p[:, :, d]
        cpa_full[:, 6 + d, :] = q[:, :, d]
    cpa_full[:, 9, :] = t_hi
    cpa_full[:, 10, :] = t_mid
    cpa_full[:, 11, :] = t_lo
    cpa_full[:, 12, :] = ones_m
    cpa_full[:, 13, :] = ones_m
    cpa_full[:, 14, :] = ones_m

    # stacked stationary: rows 32b..32b+KD, cols h*128..
    cpa = np.zeros((128, HALVES * 128), BF16)
    for b in range(B):
        for h in range(HALVES):
            cpa[32 * b : 32 * b + KD, h * 128 : (h + 1) * 128] = \
                cpa_full[b, :, h * 128 : (h + 1) * 128]

    wps = np.zeros((NBLK, 128, JP), np.float32)  # cast to bf16 at the end
    c32 = c.astype(np.float32)
    a5 = np.stack(
        [c32[:, :, 0], c32[:, :, 1], c32[:, :, 2],
         (c32 * c32).sum(-1), np.ones((B, M), np.float32)],
        axis=1,
    )  # (B, 5, M)
    for b in range(B):
        for h in range(HALVES):
            l = 2 * b + h
            msl = slice(h * 128, (h + 1) * 128)
            for k in range(5):
                for d in range(3):
                    j = k * 12 + b * 3 + d
                    wps[l, :, j] = 0.5 * a5[b, k, msl] * w[b, msl, d]

    rmat = np.zeros((128, 64), np.float32)
    for j in range(J):
        rmat[j, j % 12] = 1.0
        rmat[64 + j, j % 12] = 1.0

    # ---- dense-point side (per core) ----
    u_all = x.astype(BF16)
    v_all = (x - u_all.astype(np.float64)).astype(BF16)
    s_all = (x * x).sum(-1)

    in_maps = []
    for core in range(NCORES):
        csl = slice(core * NS, (core + 1) * NS)
        u = u_all[:, csl, :].astype(np.float32)
        v = v_all[:, csl, :].astype(np.float32)
        s_hi, s_mid, s_lo = _split3(s_all[:, csl])
        ones_n = np.ones((B, NS), BF16)

        daug_b = np.empty((B, KD, NS), BF16)
        for d in range(3):
            daug_b[:, d, :] = (-2.0 * u[:, :, d]).astype(BF16)
            daug_b[:, 3 + d, :] = (-2.0 * v[:, :, d]).astype(BF16)
            daug_b[:, 6 + d, :] = (-2.0 * u[:, :, d]).astype(BF16)
        daug_b[:, 9, :] = ones_n
        daug_b[:, 10, :] = ones_n
        daug_b[:, 11, :] = ones_n
        daug_b[:, 12, :] = s_hi
        daug_b[:, 13, :] = s_mid
        daug_b[:, 14, :] = s_lo

        daug = np.zeros((128, NS), BF16)
        for b in range(B):
            daug[32 * b : 32 * b + KD] = daug_b[b]

        xs = x[:, csl, :].astype(np.float32)
        baug5 = np.stack(
            [-2.0 * xs[:, :, 0], -2.0 * xs[:, :, 1], -2.0 * xs[:, :, 2],
             np.ones((B, NS), np.float32), (xs * xs).sum(-1)],
            axis=1,
        )  # (B, 5, NS)
        bc = np.empty((J, NS), np.float32)
        for k in range(5):
            for b in range(B):
                for d in range(3):
                    bc[k * 12 + b * 3 + d] = baug5[b, k]

        in_maps.append(
            {
                "daug": daug,
                "bcs": bc,
                "cpa": cpa,
                "wps": wps,
                "rmat": rmat,
            }
        )
    return in_maps


def _assemble(results):
    out = np.empty((B, N, 3), np.float32)
    for core, r in enumerate(results):
        o = r["outb"]  # (12, NS) rows b*3+d
        out[:, core * NS : (core + 1) * NS, :] = (
            o.reshape(B, 3, NS).transpose(0, 2, 1)
        )
    return out


def kernel(sparse_disp, original_cp, original_dense):
    global _compiled
    from concourse.bass_utils import run_bass_kernel_spmd

    if _compiled is None:
        _compiled = _build_nc()
    in_maps = _host_prep(sparse_disp, original_cp, original_dense)
    res = run_bass_kernel_spmd(_compiled, in_maps, core_ids=list(range(NCORES)))
    return _assemble(res.results)
